# revision 10
# baseline (speedup 1.0000x reference)
"""Trainium2 Bass kernel for nn_EndPointSpline.

Reference computation (per batch column b, feature d):
    xt = concat([x0, knots_b, x1])           # [T=128] knot values
    t  = spline_discr[:, b]                  # [T] sorted, t[0]=0, t[-1]=1
    vel[j] = (xt[j+1]-xt[j]) / (t[j+1]-t[j]+1e-10)
    left(q) = searchsorted(t[1:], q, 'left') clipped to [0, T-2]
    y(q) = xt[left] + vel[left] * (q - t[left])

Kernel strategy (data-parallel over B across 8 cores, 16 columns/core):
  Piecewise-linear interpolation in *integrated* form, normalized per
  segment so weights live in [0, 1]:
      y(q) = x0 + sum_j dxt[j] * clamp((q - t[j]) / dt'[j], 0, 1)
  with dxt[j] = xt[j+1]-xt[j] and dt'[j] = t[j+1]-t[j]+1e-10.  The
  weight matrix W'[i,q] is built in TWO fused DVE tensor_scalar ops
  (both 2x_2P fp32) over [128, 2048]:
      w = (q - tsh[i]) * rsh[i]     (row-0 sentinels tsh=-1, rsh=1)
      w = min(max(w, 0), 1)         (immediates; row 0 == 1 exactly)
  Saturated weights are EXACTLY 1.0, so fp16 W'/A cost only one
  boundary term of fp16 rounding per query: measured rel err 2.8e-3
  against the 2e-2 budget.  Row 0 of A holds x0; rows 1..127 hold dxt.
  One K=128 fp16 matmul per query tile gathers + lerps in one pass.

  The kernel is DMA-fabric-bound (436 GB/s SBUF-AXI per core), so all
  dtypes are chosen to minimize HBM bytes: A is fp16 (2 MiB/core), the
  device output is fp16 [BL, Q, D] (32 MiB/core, host upcasts to f32).
  PSUM (f32) evacuation is split DVE/ACT 5:11 (balancing the two DVE
  weight ops) and fully hides under the output DMA.

  Host-side marshalling: queries are permuted so each output partition
  writes a 16KB-contiguous DRAM run, giving ONE 2MiB output DMA per b
  (the kernel writes the output in ORIGINAL query order; only the
  compute order is permuted).
"""

import numpy as np

Q, B, T, D = 2048, 128, 128, 512
NCORES = 8
BL = B // NCORES          # 16 batch columns per core
K = T - 1                 # 127 segments
NQT = Q // 128            # 16 query tiles of 128
PS_QT = 4                 # query tiles per PSUM tile (4 banks)
NBLK = NQT // PS_QT       # evac blocks per b

_PROGRAM = None


def permute_queries(query_t):
    """qperm[k*128 + p] = query_t[p*NQT + k] so that output partition p
    holds queries p*16..p*16+15 (a 16KB fp16 DRAM run)."""
    a = np.asarray(query_t, dtype=np.float32).reshape(128, NQT)
    return np.ascontiguousarray(a.T.reshape(-1))


def host_prep(query_t, knots, x0, x1, spline_discr):
    """Everything that is shared across cores (f64 math, cast down)."""
    xt = np.concatenate(
        [
            np.asarray(x0, dtype=np.float32).transpose(1, 0, 2),
            np.asarray(knots, dtype=np.float32),
            np.asarray(x1, dtype=np.float32).transpose(1, 0, 2),
        ],
        axis=1,
    )                                                      # [B, T, D] f32
    t64 = np.asarray(spline_discr, dtype=np.float32).astype(np.float64)
    dtp64 = (t64[1:] - t64[:-1]) + 1e-10                   # [K, B]
    A = np.empty((B, T, D), np.float16)
    A[:, 0] = xt[:, 0]
    A[:, 1:] = xt[:, 1:, :] - xt[:, :-1, :]                # dxt

    # shifted t / reciprocal-dt with row-0 sentinels (-1 / +1 -> row-0
    # weight == clamp(q+1, 0, 1) == 1)
    tsh = np.empty((T, B), np.float32)
    tsh[0] = -1.0
    tsh[1:] = np.asarray(spline_discr, dtype=np.float32)[:K]
    rsh = np.empty((T, B), np.float32)
    rsh[0] = 1.0
    rsh[1:] = (1.0 / dtp64).astype(np.float32)
    return A, tsh, rsh


def make_core_inputs(qperm, A, tsh, rsh, core):
    s = slice(core * BL, (core + 1) * BL)
    return {
        "query_t": qperm,
        "amat": np.ascontiguousarray(A[s]),
        "tsh": np.ascontiguousarray(tsh[:, s]),
        "rsh": np.ascontiguousarray(rsh[:, s]),
    }


def _build_program(reps=1):
    import concourse.tile as tile
    from concourse import bacc, mybir

    f32 = mybir.dt.float32
    f16 = mybir.dt.float16
    Alu = mybir.AluOpType

    nc = bacc.Bacc("TRN2", target_bir_lowering=False, debug=False)

    q_d = nc.dram_tensor("query_t", [Q], f32, kind="ExternalInput").ap()
    a_d = nc.dram_tensor("amat", [BL, T, D], f16, kind="ExternalInput").ap()
    tsh_d = nc.dram_tensor("tsh", [T, BL], f32, kind="ExternalInput").ap()
    rsh_d = nc.dram_tensor("rsh", [T, BL], f32, kind="ExternalInput").ap()
    out_d = nc.dram_tensor("out", [BL, Q, D], f16, kind="ExternalOutput").ap()

    with tile.TileContext(nc) as tc:
        with (
            tc.tile_pool(name="const", bufs=1) as cpool,
            tc.tile_pool(name="apool", bufs=3) as apool,
            tc.tile_pool(name="wpool", bufs=2) as wpool,
            tc.tile_pool(name="outsb", bufs=4) as outpool,
            tc.tile_pool(name="psum", bufs=2, space="PSUM") as pspool,
        ):
            # --- per-core constants ---
            qb = cpool.tile([T, Q], f32)
            nc.scalar.dma_start(out=qb[:], in_=q_d.partition_broadcast(T))
            tshs = cpool.tile([T, BL], f32)
            nc.scalar.dma_start(out=tshs[:], in_=tsh_d[:, :])
            rshs = cpool.tile([T, BL], f32)
            nc.scalar.dma_start(out=rshs[:], in_=rsh_d[:, :])

            ecnt = 0
            for rep in range(reps):
                for b in range(BL):
                    af = apool.tile([T, D], f16)
                    nc.gpsimd.dma_start(out=af[:], in_=a_d[b, :, :])

                    # normalized clamp weights in two fused DVE ops; wt is
                    # fp16 so the clamp op runs 4x_2P (overflow -> +-inf
                    # clamps to exactly 1/0)
                    wt = wpool.tile([T, Q], f16, tag="wtmp")
                    nc.vector.tensor_scalar(
                        out=wt[:], in0=qb[:], scalar1=tshs[:, b : b + 1],
                        scalar2=rshs[:, b : b + 1],
                        op0=Alu.subtract, op1=Alu.mult,
                    )
                    w = wpool.tile([T, Q], f16, tag="wf16")
                    nc.vector.tensor_scalar(
                        out=w[:], in0=wt[:], scalar1=0.0, scalar2=1.0,
                        op0=Alu.max, op1=Alu.min,
                    )

                    osb = outpool.tile([128, NQT * D], f16)
                    for blk in range(NBLK):
                        ps = pspool.tile([128, PS_QT * D], f32)
                        for k2 in range(PS_QT):
                            qt = blk * PS_QT + k2
                            sl = slice(qt * 128, (qt + 1) * 128)
                            nc.tensor.matmul(
                                ps[:, k2 * D : (k2 + 1) * D],
                                lhsT=w[:, sl], rhs=af[:],
                                start=True, stop=True,
                            )
                        dst = osb[:, blk * PS_QT * D : (blk + 1) * PS_QT * D]
                        if ecnt % 16 in (0, 3, 6, 9, 12):
                            nc.vector.tensor_copy(out=dst, in_=ps[:])
                        else:
                            nc.scalar.copy(out=dst, in_=ps[:])
                        ecnt += 1
                    # one 2MiB DMA per b; each partition writes a 16KB run
                    dview = out_d[b].rearrange("(p c) d -> p (c d)", p=128)
                    nc.sync.dma_start(out=dview, in_=osb[:])
    nc.finalize()
    return nc


def _get_program(reps=1):
    global _PROGRAM
    if _PROGRAM is None:
        _PROGRAM = {}
    if reps not in _PROGRAM:
        _PROGRAM[reps] = _build_program(reps)
    return _PROGRAM[reps]


def kernel(query_t, knots, x0, x1, spline_discr, _trace=False, **_trace_kwargs):
    from concourse.bass_utils import run_bass_kernel_spmd

    qperm = permute_queries(query_t)
    A, tsh, rsh = host_prep(query_t, knots, x0, x1, spline_discr)

    nc = _get_program()
    in_maps = [
        make_core_inputs(qperm, A, tsh, rsh, c) for c in range(NCORES)
    ]
    res = run_bass_kernel_spmd(
        nc, in_maps, core_ids=list(range(NCORES)), trace=_trace, **_trace_kwargs
    )
    out = np.concatenate(
        [r["out"].astype(np.float32) for r in res.results], axis=0
    )
    if _trace:
        return out, res
    return out
